# revision 27
# baseline (speedup 1.0000x reference)
"""Trainium2 Bass kernel for nn_AttentionModel (B=4, S=4096, E=2048) on 8 cores.

Sharding: data-parallel over batch B (4 pairs of cores) x tensor-parallel over
the E dim (2 cores per pair). Core c handles batch b=c//2, half h=c%2:
  phase 1kq: computes its OWN half of kT [S, EH] and qT [S, EH]
  phase 1v:  computes its OWN half of v [EH, S]
  pair AllGathers exchange the halves (k in 2 chunks issued mid-phase-1kq,
             v in 2 chunks issued mid-phase-1v) so each core holds full
             kT [S, E] and v [E, S] without duplicating projection FLOPs,
             and the collective wire time hides under compute
  phase 2:   scoresT [f, e_h] = kT-tile.T @ qT directly in transposed layout
             (stationary k tiles streamed per f-tile, moving q resident)
             -> no PE transposes needed; softmax = plain exp (scores max ~15,
             f32 psum, no max shift), attnT bf16 to DRAM; denominators via
             all-ones matmul; 1/sum applied per-partition at phase-4 eviction
  phase 4:   outT row block = attnT-tile.T @ v, scaled by 1/sum at eviction.

All matmul operands bf16 (same PE rate as fp32r, half the DMA/SBUF traffic),
fp32 accumulation. k/q biases are added by the vector engine during PSUM
eviction (bias pre-replicated across partitions on the host); v bias via
per-partition activation bias. The 1/sqrt(E) score scale is folded into
Wq/bq on the host.

DMA issue engines are chosen so loads never queue behind compute-dependent
stores: big loads on sync, x/q loads on scalar/gpsimd, stores paired after
the op that produced their data, collectives on gpsimd.
"""

import sys

sys.path.insert(0, "/opt/trn_rl_repo")

from contextlib import ExitStack

import ml_dtypes
import numpy as np

import concourse.bass as bass
import concourse.mybir as mybir
import concourse.tile as tile
from concourse import bacc
from concourse.bass_utils import run_bass_kernel_spmd

bf16 = mybir.dt.bfloat16
f32 = mybir.dt.float32
bfnp = ml_dtypes.bfloat16

B, S, E = 4, 4096, 2048
EH = E // 2          # per-core half of the E dim (q/k cols, v rows, out rows)
N = 512              # moving free-dim per matmul (one PSUM bank of f32)
SKT = S // 128       # 32 s-tiles
EKT = E // 128       # 16 e-tiles (also: f-tiles over full E)
FH = EH // 128       # 8 f-tiles per half
N_CORES = 8
RG = [[0, 1], [2, 3], [4, 5], [6, 7]]  # pairs share a batch

Exp = mybir.ActivationFunctionType.Exp
Identity = mybir.ActivationFunctionType.Identity
ADD = mybir.AluOpType.add


def build_kernel():
    nc = bacc.Bacc("TRN2", debug=False, target_bir_lowering=False)

    # x^T tiles: xtt[st][p=e_in, kt, s_in] = x[st*128+s_in, kt*128+p]
    xtt = nc.dram_tensor("xtt", [SKT, 128, EKT, 128], bf16, kind="ExternalInput")
    # x^T rows for phase 1v: xte[sh][ekt][p=e_in][s] = xT[ekt*128+p, sh*2048+s]
    xte = nc.dram_tensor("xte", [2, EKT, 128, S // 2], bf16, kind="ExternalInput")
    wqk = nc.dram_tensor("wqk", [E, E], bf16, kind="ExternalInput")   # [WkT_h | WqT_h*sc]
    bkq = nc.dram_tensor("bkq", [128, E], bf16, kind="ExternalInput")  # replicated rows
    wv = nc.dram_tensor("wv", [FH, E, 128], bf16, kind="ExternalInput")  # WvT_h f-tiled
    bv = nc.dram_tensor("bv", [128, FH], f32, kind="ExternalInput")   # bv_h per f-tile
    ones_d = nc.dram_tensor("ones", [128, 128], bf16, kind="ExternalInput")
    outt = nc.dram_tensor("outt", [EH, S], bf16, kind="ExternalOutput")

    with tile.TileContext(nc) as tc, ExitStack() as ctx:
        dram = ctx.enter_context(tc.tile_pool(name="dram", bufs=1, space="DRAM"))
        k_h = dram.tile([2, S // 2, EH], bf16)              # own kT cols, 2 chunks
        q_d = dram.tile([S, EH], bf16)                      # own qT cols
        v_h = dram.tile([2, EH, S // 2], bf16)              # own v rows, 2 s-chunks
        sums_d = dram.tile([1, EH], f32)                    # softmax denominators
        k_g = dram.tile([2, 2, S // 2, EH], bf16)           # [chunk][slot]
        v_g = dram.tile([2, 2, EH, S // 2], bf16)           # [s-chunk][slot]

        const = ctx.enter_context(tc.tile_pool(name="const", bufs=1))
        ones_sb = const.tile([128, 128], bf16)
        bkq_sb = const.tile([128, E], bf16)
        bv_sb = const.tile([128, FH], f32)
        rsum_sb = const.tile([128, FH], f32)
        rsum_tmp = const.tile([128, FH], f32)

        # attnT stays SBUF-resident from phase 2 through phase 4
        p_at = ctx.enter_context(tc.tile_pool(name="p_at", bufs=1))
        at_all = p_at.tile([128, EKT, EH], bf16)

        # phase-1v pools live from kernel start (prefetch during 1kq);
        # sb_stack closes them (and the phase-2 pools) before phase 4
        sb_stack = ExitStack()
        p_wv = sb_stack.enter_context(tc.tile_pool(name="pv_w", bufs=1))
        p_xh = sb_stack.enter_context(tc.tile_pool(name="pv_x", bufs=2))
        p_ve = sb_stack.enter_context(tc.tile_pool(name="pv_e", bufs=3))

        ps_stack = ExitStack()
        p_ps = ps_stack.enter_context(
            tc.tile_pool(name="ps_big", bufs=2, space="PSUM")
        )

        # ---- Phase 1kq: [kT_h | qT_h] = x^T-tiles.T @ [WkT | WqT] ----
        with (
            tc.tile_pool(name="p1_w", bufs=1) as p_w,
            tc.tile_pool(name="p1_x", bufs=3) as p_x,
            tc.tile_pool(name="p1_e", bufs=2) as p_e,
        ):
            w_sb = p_w.tile([128, EKT, E], bf16)
            for ekt in range(EKT):
                nc.sync.dma_start(
                    w_sb[:, ekt, :], wqk[ekt * 128:(ekt + 1) * 128, :]
                )
            nc.sync.dma_start(bkq_sb[:, :], bkq[:, :])
            wv_sb = p_wv.tile([128, FH, EKT, 128], bf16)
            for ft in range(FH):
                nc.sync.dma_start(
                    wv_sb[:, ft], wv[ft].rearrange("(kt p) f -> p kt f", p=128)
                )
            nc.sync.dma_start(bv_sb[:, :], bv[:, :])
            nc.sync.dma_start(ones_sb[:, :], ones_d[:, :])
            for st in range(SKT):
                xtc = p_x.tile([128, EKT, 128], bf16, tag="xtc")
                nc.scalar.dma_start(xtc[:, :, :], xtt[st])
                ps = p_ps.tile([128, E], f32, tag="ps")
                for ekt in range(EKT):
                    for fc in range(E // N):
                        nc.tensor.matmul(
                            ps[:, fc * N:(fc + 1) * N],
                            xtc[:, ekt, :],
                            w_sb[:, ekt, fc * N:(fc + 1) * N],
                            start=(ekt == 0),
                            stop=(ekt == EKT - 1),
                        )
                kq = p_e.tile([128, E], bf16, tag="kq")
                nc.vector.tensor_tensor(
                    kq[:, :], ps[:, :], bkq_sb[:, :], op=ADD
                )
                ck, crow = st // (SKT // 2), st % (SKT // 2)
                rows = slice(crow * 128, (crow + 1) * 128)
                nc.gpsimd.dma_start(k_h[ck, rows, :], kq[:, 0:EH])
                nc.gpsimd.dma_start(
                    q_d[st * 128:(st + 1) * 128, :], kq[:, EH:E]
                )
                if st in (SKT // 2 - 1, SKT - 1):
                    # AllGather this half of k as soon as it completes
                    nc.gpsimd.collective_compute(
                        "AllGather",
                        mybir.AluOpType.bypass,
                        replica_groups=RG,
                        ins=[k_h[ck].opt()],
                        outs=[k_g[ck].opt()],
                    )

        # phase-2 SBUF pools: allocated now (p1 pools freed) so q and the
        # first k-tiles load during phase 1v, under compute
        p_q = sb_stack.enter_context(tc.tile_pool(name="p2_q", bufs=1))
        p_kb = sb_stack.enter_context(tc.tile_pool(name="p2_k", bufs=3))
        p_s = sb_stack.enter_context(tc.tile_pool(name="p2_s", bufs=1))
        qt = p_q.tile([128, SKT, EH], bf16)

        # ---- Phase 1v: v_h [f_local, s] = WvT-tiles.T @ x^T rows ----
        SQ = N
        for sq in range(S // SQ):
            sh, sc_ = sq // 4, sq % 4
            xth = p_xh.tile([128, EKT, SQ], bf16, tag="xth")
            for ekt in range(EKT):
                nc.sync.dma_start(
                    xth[:, ekt, :],
                    xte[sh, ekt, :, sc_ * SQ:(sc_ + 1) * SQ],
                )
            for ft in range(FH):
                # full-size tile, same tag as 1kq -> same 2 psum bufs
                psv = p_ps.tile([128, E], f32, tag="ps")
                for ekt in range(EKT):
                    nc.tensor.matmul(
                        psv[:, 0:SQ],
                        wv_sb[:, ft, ekt],
                        xth[:, ekt, :],
                        start=(ekt == 0),
                        stop=(ekt == EKT - 1),
                    )
                vsb = p_ve.tile([128, SQ], bf16, tag="vsb")
                nc.scalar.activation(
                    vsb[:, :], psv[:, 0:SQ], Identity,
                    bias=bv_sb[:, ft:ft + 1], scale=1.0,
                )
                nc.scalar.dma_start(
                    v_h[sh, ft * 128:(ft + 1) * 128, sc_ * SQ:(sc_ + 1) * SQ],
                    vsb[:, :],
                )
            if sq in (3, 7):
                # AllGather this s-half of v as soon as it completes
                nc.gpsimd.collective_compute(
                    "AllGather",
                    mybir.AluOpType.bypass,
                    replica_groups=RG,
                    ins=[v_h[sh].opt()],
                    outs=[v_g[sh].opt()],
                )

        for skt in range(SKT):
            nc.sync.dma_start(
                qt[:, skt, :], q_d[skt * 128:(skt + 1) * 128, :]
            )

        ps_stack.close()  # free phase-1 psum banks for phase 2

        # ---- Phase 2: attnT[f, e_h] = exp(kT-tiles.T @ qT); denominators ----
        with (
            tc.tile_pool(name="p2_ps", bufs=3, space="PSUM") as p_sc,
            tc.tile_pool(name="p2_sm", bufs=1, space="PSUM") as p_sm,
        ):
            sums_ps = p_sm.tile([128, EH], f32)
            pending = None  # software pipeline: sums matmuls lag one fkt
            for fkt in range(EKT):
                sl, fcol = fkt // FH, (fkt % FH) * 128
                kb = p_kb.tile([128, SKT, 128], bf16, tag="kb")
                for ck in range(2):
                    nc.sync.dma_start(
                        kb[:, ck * (SKT // 2):(ck + 1) * (SKT // 2), :],
                        k_g[ck, sl, :, fcol:fcol + 128].rearrange(
                            "(skt p) f -> p skt f", p=128
                        ),
                    )
                scp = p_sc.tile([128, EH], f32, tag="scp")
                for skt in range(SKT):
                    for ec in range(EH // N):
                        nc.tensor.matmul(
                            scp[:, ec * N:(ec + 1) * N],
                            kb[:, skt, :],
                            qt[:, skt, ec * N:(ec + 1) * N],
                            start=(skt == 0),
                            stop=(skt == SKT - 1),
                        )
                if pending is not None:
                    pf = pending
                    for ec in range(EH // N):
                        nc.tensor.matmul(
                            sums_ps[:, ec * N:(ec + 1) * N],
                            ones_sb[:, :],
                            at_all[:, pf, ec * N:(ec + 1) * N],
                            start=(pf == 0),
                            stop=False,
                        )
                nc.scalar.activation(at_all[:, fkt, :], scp[:, :], Exp)
                pending = fkt
            pf = pending
            for ec in range(EH // N):
                nc.tensor.matmul(
                    sums_ps[:, ec * N:(ec + 1) * N],
                    ones_sb[:, :],
                    at_all[:, pf, ec * N:(ec + 1) * N],
                    start=False,
                    stop=(ec == EH // N - 1),
                )
            # denominators -> reciprocal in [p, et] layout via DRAM bounce
            sums_row = p_s.tile([1, EH], f32)
            nc.vector.tensor_copy(sums_row[:, :], sums_ps[0:1, :])
            nc.sync.dma_start(sums_d[:, :], sums_row[:, :])
            nc.sync.dma_start(
                rsum_tmp[:, :],
                sums_d[:, :].rearrange("o (et p) -> (o p) et", p=128),
            )
            nc.vector.reciprocal(rsum_sb[:, :], rsum_tmp[:, :])

        sb_stack.close()  # free phase-1v/2 SBUF for phase 4

        # ---- Phase 4: outT rows = attnT-tiles.T @ v, * rsum at eviction ----
        with (
            tc.tile_pool(name="p4_v", bufs=2) as p_v,
            tc.tile_pool(name="p4_o", bufs=3) as p_o,
            tc.tile_pool(name="p4_ps", bufs=3, space="PSUM") as p_ps4,
        ):
            SB = 1024
            for sb in range(S // SB):
                sck, scol = sb // 2, (sb % 2) * SB
                vb = p_v.tile([128, EKT, SB], bf16, tag="vb")
                for fkt in range(EKT):
                    sl, fl = fkt // FH, fkt % FH
                    nc.sync.dma_start(
                        vb[:, fkt, :],
                        v_g[sck, sl, fl * 128:(fl + 1) * 128,
                            scol:scol + SB],
                    )
                for et in range(FH):
                    ps4 = p_ps4.tile([128, SB], f32, tag="ps4")
                    for fkt in range(EKT):
                        for sc in range(SB // N):
                            nc.tensor.matmul(
                                ps4[:, sc * N:(sc + 1) * N],
                                at_all[:, fkt, et * 128:(et + 1) * 128],
                                vb[:, fkt, sc * N:(sc + 1) * N],
                                start=(fkt == 0),
                                stop=(fkt == EKT - 1),
                            )
                    osb = p_o.tile([128, SB], bf16, tag="osb")
                    nc.scalar.activation(
                        osb[:, :], ps4[:, :], Identity,
                        scale=rsum_sb[:, et:et + 1],
                    )
                    nc.scalar.dma_start(
                        outt[et * 128:(et + 1) * 128, sb * SB:(sb + 1) * SB],
                        osb[:, :],
                    )

    nc.compile()
    return nc


_NC_CACHE = {}


def _get_nc():
    if "nc" not in _NC_CACHE:
        _NC_CACHE["nc"] = build_kernel()
    return _NC_CACHE["nc"]


def make_in_maps(x, Wq, bq, Wk, bk, Wv, bv):
    sc = np.float32(1.0 / np.sqrt(E))
    wk_t = np.ascontiguousarray(Wk.T)                       # [E, E]
    wq_t = np.ascontiguousarray(Wq.T) * sc
    wv_t = np.ascontiguousarray(Wv.T)
    ones = np.ones((128, 128), bfnp)
    in_maps = []
    for c in range(N_CORES):
        b, h = c // 2, c % 2
        xb = x[b]                                           # [S, E]
        cols = slice(h * EH, (h + 1) * EH)
        xtt = np.ascontiguousarray(
            xb.reshape(SKT, 128, EKT, 128).transpose(0, 3, 2, 1)
        ).astype(bfnp)                                      # [st, e_in, kt, s_in]
        xte = np.ascontiguousarray(
            xb.T.reshape(EKT, 128, 2, S // 2).transpose(2, 0, 1, 3)
        ).astype(bfnp)                                      # [sh, ekt, p, s]
        wqk = np.concatenate([wk_t[:, cols], wq_t[:, cols]], axis=1).astype(bfnp)
        bkq_row = np.concatenate([bk[cols], bq[cols] * sc])[None, :]
        bkq = np.broadcast_to(bkq_row, (128, E)).astype(bfnp)
        wvh = np.ascontiguousarray(
            wv_t[:, cols].reshape(E, FH, 128).transpose(1, 0, 2)
        ).astype(bfnp)                                      # [FH, E, 128]
        bvh = np.ascontiguousarray(bv[cols].reshape(FH, 128).T).astype(np.float32)
        in_maps.append({
            "xtt": xtt,
            "xte": xte,
            "wqk": np.ascontiguousarray(wqk),
            "bkq": np.ascontiguousarray(bkq),
            "wv": wvh,
            "bv": bvh,
            "ones": ones,
        })
    return in_maps


def run(in_maps, trace=False, **kwargs):
    nc = _get_nc()
    return run_bass_kernel_spmd(
        nc, in_maps, core_ids=list(range(N_CORES)), trace=trace, **kwargs
    )


def kernel(x, Wq, bq, Wk, bk, Wv, bv):
    x = np.asarray(x, dtype=np.float32)
    in_maps = make_in_maps(
        x,
        np.asarray(Wq, np.float32), np.asarray(bq, np.float32),
        np.asarray(Wk, np.float32), np.asarray(bk, np.float32),
        np.asarray(Wv, np.float32), np.asarray(bv, np.float32),
    )
    res = run(in_maps, trace=False)
    out = np.empty((B, E, S), dtype=np.float32)
    for c in range(N_CORES):
        b, h = c // 2, c % 2
        out[b, h * EH:(h + 1) * EH, :] = res.results[c]["outt"]
    return out


# revision 28
# speedup vs baseline: 1.1915x; 1.1915x over previous
"""Trainium2 Bass kernel for nn_AttentionModel (B=4, S=4096, E=2048) on 8 cores.

Sharding: data-parallel over batch B (4 pairs of cores) x tensor-parallel over
the E dim (2 cores per pair). Core c handles batch b=c//2, half h=c%2:
  phase 1kq: computes its OWN half of kT [S, EH] and qT [S, EH]
  phase 1v:  computes its OWN half of v [EH, S]
  pair AllGathers exchange the halves (k in 2 chunks issued mid-phase-1kq,
             v in 2 chunks issued mid-phase-1v) so each core holds full
             kT [S, E] and v [E, S] without duplicating projection FLOPs,
             and the collective wire time hides under compute
  phase 2:   scoresT [f, e_h] = kT-tile.T @ qT directly in transposed layout
             (stationary k tiles streamed per f-tile, moving q resident)
             -> no PE transposes needed; softmax = plain exp (scores max ~15,
             f32 psum, no max shift), attnT kept in SBUF through phase 4; denominators via
             all-ones matmul; 1/sum applied per-partition at phase-4 eviction
  phase 4:   outT row block = attnT-tile.T @ v, scaled by 1/sum at eviction.

All matmul operands bf16 (same PE rate as fp32r, half the DMA/SBUF traffic),
fp32 accumulation. k/q biases are added by the vector engine during PSUM
eviction (bias pre-replicated across partitions on the host); v bias via
per-partition activation bias. The 1/sqrt(E) score scale is folded into
Wq/bq on the host.

DMA issue engines are chosen so loads never queue behind compute-dependent
stores: big loads on sync, x/q loads on scalar/gpsimd, stores paired after
the op that produced their data, collectives on gpsimd.
"""

import sys

sys.path.insert(0, "/opt/trn_rl_repo")

from contextlib import ExitStack

import ml_dtypes
import numpy as np

import concourse.bass as bass
import concourse.mybir as mybir
import concourse.tile as tile
from concourse import bacc
from concourse.bass_utils import run_bass_kernel_spmd

bf16 = mybir.dt.bfloat16
f32 = mybir.dt.float32
bfnp = ml_dtypes.bfloat16

B, S, E = 4, 4096, 2048
EH = E // 2          # per-core half of the E dim (q/k cols, v rows, out rows)
N = 512              # moving free-dim per matmul (one PSUM bank of f32)
SKT = S // 128       # 32 s-tiles
EKT = E // 128       # 16 e-tiles (also: f-tiles over full E)
FH = EH // 128       # 8 f-tiles per half
N_CORES = 8
RG = [[0, 1], [2, 3], [4, 5], [6, 7]]  # pairs share a batch

Exp = mybir.ActivationFunctionType.Exp
Identity = mybir.ActivationFunctionType.Identity
ADD = mybir.AluOpType.add


def build_kernel():
    nc = bacc.Bacc("TRN2", debug=False, target_bir_lowering=False)

    # x^T tiles: xtt[st][p=e_in, kt, s_in] = x[st*128+s_in, kt*128+p]
    xtt = nc.dram_tensor("xtt", [SKT, 128, EKT, 128], bf16, kind="ExternalInput")
    # x^T rows for phase 1v: xte[sh][ekt][p=e_in][s] = xT[ekt*128+p, sh*2048+s]
    xte = nc.dram_tensor("xte", [2, EKT, 128, S // 2], bf16, kind="ExternalInput")
    wqk = nc.dram_tensor("wqk", [E, E], bf16, kind="ExternalInput")   # [WkT_h | WqT_h*sc]
    bkq = nc.dram_tensor("bkq", [128, E], bf16, kind="ExternalInput")  # replicated rows
    wv = nc.dram_tensor("wv", [FH, E, 128], bf16, kind="ExternalInput")  # WvT_h f-tiled
    bv = nc.dram_tensor("bv", [128, FH], f32, kind="ExternalInput")   # bv_h per f-tile
    ones_d = nc.dram_tensor("ones", [128, 128], bf16, kind="ExternalInput")
    outt = nc.dram_tensor("outt", [EH, S], bf16, kind="ExternalOutput")

    with tile.TileContext(nc) as tc, ExitStack() as ctx:
        dram = ctx.enter_context(tc.tile_pool(name="dram", bufs=1, space="DRAM"))
        k_h = dram.tile([2, S // 2, EH], bf16)              # own kT cols, 2 chunks
        q_d = dram.tile([S, EH], bf16)                      # own qT cols
        v_h = dram.tile([2, EH, S // 2], bf16)              # own v rows, 2 s-chunks
        sums_d = dram.tile([1, EH], f32)                    # softmax denominators
        k_g = dram.tile([2, 2, S // 2, EH], bf16)           # [chunk][slot]
        v_g = dram.tile([2, 2, EH, S // 2], bf16)           # [s-chunk][slot]

        const = ctx.enter_context(tc.tile_pool(name="const", bufs=1))
        ones_sb = const.tile([128, 128], bf16)
        bkq_sb = const.tile([128, E], bf16)
        bv_sb = const.tile([128, FH], f32)
        rsum_sb = const.tile([128, FH], f32)
        rsum_tmp = const.tile([128, FH], f32)

        # attnT stays SBUF-resident from phase 2 through phase 4
        p_at = ctx.enter_context(tc.tile_pool(name="p_at", bufs=1))
        at_all = p_at.tile([128, EKT, EH], bf16)

        # phase-1v pools live from kernel start (prefetch during 1kq);
        # sb_stack closes them (and the phase-2 pools) before phase 4
        sb_stack = ExitStack()
        p_wv = sb_stack.enter_context(tc.tile_pool(name="pv_w", bufs=1))
        p_xh = sb_stack.enter_context(tc.tile_pool(name="pv_x", bufs=2))
        p_ve = sb_stack.enter_context(tc.tile_pool(name="pv_e", bufs=3))

        ps_stack = ExitStack()
        p_ps = ps_stack.enter_context(
            tc.tile_pool(name="ps_big", bufs=2, space="PSUM")
        )

        # ---- Phase 1kq: [kT_h | qT_h] = x^T-tiles.T @ [WkT | WqT] ----
        with (
            tc.tile_pool(name="p1_w", bufs=1) as p_w,
            tc.tile_pool(name="p1_x", bufs=3) as p_x,
            tc.tile_pool(name="p1_e", bufs=2) as p_e,
        ):
            w_sb = p_w.tile([128, EKT, E], bf16)
            for ekt in range(EKT):
                nc.sync.dma_start(
                    w_sb[:, ekt, :], wqk[ekt * 128:(ekt + 1) * 128, :]
                )
            nc.sync.dma_start(bkq_sb[:, :], bkq[:, :])
            wv_sb = p_wv.tile([128, FH, EKT, 128], bf16)
            for ft in range(FH):
                nc.sync.dma_start(
                    wv_sb[:, ft], wv[ft].rearrange("(kt p) f -> p kt f", p=128)
                )
            nc.sync.dma_start(bv_sb[:, :], bv[:, :])
            nc.sync.dma_start(ones_sb[:, :], ones_d[:, :])
            for st in range(SKT):
                xtc = p_x.tile([128, EKT, 128], bf16, tag="xtc")
                nc.scalar.dma_start(xtc[:, :, :], xtt[st])
                ps = p_ps.tile([128, E], f32, tag="ps")
                for ekt in range(EKT):
                    for fc in range(E // N):
                        nc.tensor.matmul(
                            ps[:, fc * N:(fc + 1) * N],
                            xtc[:, ekt, :],
                            w_sb[:, ekt, fc * N:(fc + 1) * N],
                            start=(ekt == 0),
                            stop=(ekt == EKT - 1),
                        )
                kq = p_e.tile([128, E], bf16, tag="kq")
                nc.vector.tensor_tensor(
                    kq[:, :], ps[:, :], bkq_sb[:, :], op=ADD
                )
                ck, crow = st // (SKT // 2), st % (SKT // 2)
                rows = slice(crow * 128, (crow + 1) * 128)
                nc.gpsimd.dma_start(k_h[ck, rows, :], kq[:, 0:EH])
                nc.gpsimd.dma_start(
                    q_d[st * 128:(st + 1) * 128, :], kq[:, EH:E]
                )
                if st in (SKT // 2 - 1, SKT - 1):
                    # AllGather this half of k as soon as it completes
                    nc.gpsimd.collective_compute(
                        "AllGather",
                        mybir.AluOpType.bypass,
                        replica_groups=RG,
                        ins=[k_h[ck].opt()],
                        outs=[k_g[ck].opt()],
                    )

        # phase-2 SBUF pools: allocated now (p1 pools freed) so q and the
        # first k-tiles load during phase 1v, under compute
        p_q = sb_stack.enter_context(tc.tile_pool(name="p2_q", bufs=1))
        p_kb = sb_stack.enter_context(tc.tile_pool(name="p2_k", bufs=3))
        p_s = sb_stack.enter_context(tc.tile_pool(name="p2_s", bufs=1))
        qt = p_q.tile([128, SKT, EH], bf16)
        for skt in range(SKT):
            nc.gpsimd.dma_start(
                qt[:, skt, :], q_d[skt * 128:(skt + 1) * 128, :]
            )

        # ---- Phase 1v: v_h [f_local, s] = WvT-tiles.T @ x^T rows ----
        SQ = N
        for sq in range(S // SQ):
            sh, sc_ = sq // 4, sq % 4
            xth = p_xh.tile([128, EKT, SQ], bf16, tag="xth")
            for ekt in range(EKT):
                nc.sync.dma_start(
                    xth[:, ekt, :],
                    xte[sh, ekt, :, sc_ * SQ:(sc_ + 1) * SQ],
                )
            for ft in range(FH):
                # full-size tile, same tag as 1kq -> same 2 psum bufs
                psv = p_ps.tile([128, E], f32, tag="ps")
                for ekt in range(EKT):
                    nc.tensor.matmul(
                        psv[:, 0:SQ],
                        wv_sb[:, ft, ekt],
                        xth[:, ekt, :],
                        start=(ekt == 0),
                        stop=(ekt == EKT - 1),
                    )
                vsb = p_ve.tile([128, SQ], bf16, tag="vsb")
                nc.scalar.activation(
                    vsb[:, :], psv[:, 0:SQ], Identity,
                    bias=bv_sb[:, ft:ft + 1], scale=1.0,
                )
                nc.scalar.dma_start(
                    v_h[sh, ft * 128:(ft + 1) * 128, sc_ * SQ:(sc_ + 1) * SQ],
                    vsb[:, :],
                )
            if sq in (3, 7):
                # AllGather this s-half of v as soon as it completes
                nc.gpsimd.collective_compute(
                    "AllGather",
                    mybir.AluOpType.bypass,
                    replica_groups=RG,
                    ins=[v_h[sh].opt()],
                    outs=[v_g[sh].opt()],
                )

        ps_stack.close()  # free phase-1 psum banks for phase 2

        # ---- Phase 2: attnT[f, e_h] = exp(kT-tiles.T @ qT); denominators ----
        with (
            tc.tile_pool(name="p2_ps", bufs=3, space="PSUM") as p_sc,
            tc.tile_pool(name="p2_sm", bufs=1, space="PSUM") as p_sm,
        ):
            sums_ps = p_sm.tile([128, EH], f32)
            pending = None  # software pipeline: sums matmuls lag one fkt
            for fkt in range(EKT):
                sl, fcol = fkt // FH, (fkt % FH) * 128
                kb = p_kb.tile([128, SKT, 128], bf16, tag="kb")
                for ck in range(2):
                    nc.sync.dma_start(
                        kb[:, ck * (SKT // 2):(ck + 1) * (SKT // 2), :],
                        k_g[ck, sl, :, fcol:fcol + 128].rearrange(
                            "(skt p) f -> p skt f", p=128
                        ),
                    )
                scp = p_sc.tile([128, EH], f32, tag="scp")
                for skt in range(SKT):
                    for ec in range(EH // N):
                        nc.tensor.matmul(
                            scp[:, ec * N:(ec + 1) * N],
                            kb[:, skt, :],
                            qt[:, skt, ec * N:(ec + 1) * N],
                            start=(skt == 0),
                            stop=(skt == SKT - 1),
                        )
                if pending is not None:
                    pf = pending
                    for ec in range(EH // N):
                        nc.tensor.matmul(
                            sums_ps[:, ec * N:(ec + 1) * N],
                            ones_sb[:, :],
                            at_all[:, pf, ec * N:(ec + 1) * N],
                            start=(pf == 0),
                            stop=False,
                        )
                nc.scalar.activation(at_all[:, fkt, :], scp[:, :], Exp)
                pending = fkt
            pf = pending
            for ec in range(EH // N):
                nc.tensor.matmul(
                    sums_ps[:, ec * N:(ec + 1) * N],
                    ones_sb[:, :],
                    at_all[:, pf, ec * N:(ec + 1) * N],
                    start=False,
                    stop=(ec == EH // N - 1),
                )
            # denominators -> reciprocal in [p, et] layout via DRAM bounce
            sums_row = p_s.tile([1, EH], f32)
            nc.vector.tensor_copy(sums_row[:, :], sums_ps[0:1, :])
            nc.sync.dma_start(sums_d[:, :], sums_row[:, :])
            nc.sync.dma_start(
                rsum_tmp[:, :],
                sums_d[:, :].rearrange("o (et p) -> (o p) et", p=128),
            )
            nc.vector.reciprocal(rsum_sb[:, :], rsum_tmp[:, :])

        sb_stack.close()  # free phase-1v/2 SBUF for phase 4

        # ---- Phase 4: outT rows = attnT-tiles.T @ v, * rsum at eviction ----
        with (
            tc.tile_pool(name="p4_v", bufs=2) as p_v,
            tc.tile_pool(name="p4_o", bufs=3) as p_o,
            tc.tile_pool(name="p4_ps", bufs=3, space="PSUM") as p_ps4,
        ):
            SB = 1024
            for sb in range(S // SB):
                sck, scol = sb // 2, (sb % 2) * SB
                vb = p_v.tile([128, EKT, SB], bf16, tag="vb")
                for fkt in range(EKT):
                    sl, fl = fkt // FH, fkt % FH
                    nc.sync.dma_start(
                        vb[:, fkt, :],
                        v_g[sck, sl, fl * 128:(fl + 1) * 128,
                            scol:scol + SB],
                    )
                for et in range(FH):
                    ps4 = p_ps4.tile([128, SB], f32, tag="ps4")
                    for fkt in range(EKT):
                        for sc in range(SB // N):
                            nc.tensor.matmul(
                                ps4[:, sc * N:(sc + 1) * N],
                                at_all[:, fkt, et * 128:(et + 1) * 128],
                                vb[:, fkt, sc * N:(sc + 1) * N],
                                start=(fkt == 0),
                                stop=(fkt == EKT - 1),
                            )
                    osb = p_o.tile([128, SB], bf16, tag="osb")
                    nc.scalar.activation(
                        osb[:, :], ps4[:, :], Identity,
                        scale=rsum_sb[:, et:et + 1],
                    )
                    nc.scalar.dma_start(
                        outt[et * 128:(et + 1) * 128, sb * SB:(sb + 1) * SB],
                        osb[:, :],
                    )

    nc.compile()
    return nc


_NC_CACHE = {}


def _get_nc():
    if "nc" not in _NC_CACHE:
        _NC_CACHE["nc"] = build_kernel()
    return _NC_CACHE["nc"]


def make_in_maps(x, Wq, bq, Wk, bk, Wv, bv):
    sc = np.float32(1.0 / np.sqrt(E))
    wk_t = np.ascontiguousarray(Wk.T)                       # [E, E]
    wq_t = np.ascontiguousarray(Wq.T) * sc
    wv_t = np.ascontiguousarray(Wv.T)
    ones = np.ones((128, 128), bfnp)
    in_maps = []
    for c in range(N_CORES):
        b, h = c // 2, c % 2
        xb = x[b]                                           # [S, E]
        cols = slice(h * EH, (h + 1) * EH)
        xtt = np.ascontiguousarray(
            xb.reshape(SKT, 128, EKT, 128).transpose(0, 3, 2, 1)
        ).astype(bfnp)                                      # [st, e_in, kt, s_in]
        xte = np.ascontiguousarray(
            xb.T.reshape(EKT, 128, 2, S // 2).transpose(2, 0, 1, 3)
        ).astype(bfnp)                                      # [sh, ekt, p, s]
        wqk = np.concatenate([wk_t[:, cols], wq_t[:, cols]], axis=1).astype(bfnp)
        bkq_row = np.concatenate([bk[cols], bq[cols] * sc])[None, :]
        bkq = np.broadcast_to(bkq_row, (128, E)).astype(bfnp)
        wvh = np.ascontiguousarray(
            wv_t[:, cols].reshape(E, FH, 128).transpose(1, 0, 2)
        ).astype(bfnp)                                      # [FH, E, 128]
        bvh = np.ascontiguousarray(bv[cols].reshape(FH, 128).T).astype(np.float32)
        in_maps.append({
            "xtt": xtt,
            "xte": xte,
            "wqk": np.ascontiguousarray(wqk),
            "bkq": np.ascontiguousarray(bkq),
            "wv": wvh,
            "bv": bvh,
            "ones": ones,
        })
    return in_maps


def run(in_maps, trace=False, **kwargs):
    nc = _get_nc()
    return run_bass_kernel_spmd(
        nc, in_maps, core_ids=list(range(N_CORES)), trace=trace, **kwargs
    )


def kernel(x, Wq, bq, Wk, bk, Wv, bv):
    x = np.asarray(x, dtype=np.float32)
    in_maps = make_in_maps(
        x,
        np.asarray(Wq, np.float32), np.asarray(bq, np.float32),
        np.asarray(Wk, np.float32), np.asarray(bk, np.float32),
        np.asarray(Wv, np.float32), np.asarray(bv, np.float32),
    )
    res = run(in_maps, trace=False)
    out = np.empty((B, E, S), dtype=np.float32)
    for c in range(N_CORES):
        b, h = c // 2, c % 2
        out[b, h * EH:(h + 1) * EH, :] = res.results[c]["outt"]
    return out
